# revision 14
# baseline (speedup 1.0000x reference)
"""Multi-head attention (B=4, S=2048, D=512, H=8, DH=64) on 8 TRN2 NeuronCores.

Sharding: core c handles batch b = c//2 and head-group g = c%2 (4 of the 8
heads).  Each core computes its QKV projection (columns of W_qkv for its
heads), attention for its 4 heads, and a partial output projection
(rows of W_out for its heads).  The host sums the two partials per batch
and adds the bias.

Per-core device layout (compute in bf16, fp32 PSUM accumulation):
  - host passes x[b] pre-transposed (xT [D, S]); the QKV projection then
    needs no on-device transpose: qkT[f, s] = sum_d wqk[d, f] * xT[d, s].
  - qT/kT are stored one head per 128-partition chunk with the unused 64
    partitions zeroed, so every matmul in the kernel runs in the same
    128x128 array mode (no TensorE mode-switch drains) and score matmuls
    contract over K=128 (the zero rows contribute nothing).
  - scores are computed transposed (scoresT [k, q]); exp(scale*s) is fused
    into the PSUM->SBUF copy on the Scalar engine, batched 2 PSUM banks at
    a time to amortize the ACTIVATE fixed overhead.
  - attn @ V uses V in natural [token, dh] layout augmented with a ones
    column: one PSUM accumulation produces outT_unnorm [dh, q] AND the
    softmax denominator row.
  - normalization: reciprocal of the denominator row, DMA-broadcast across
    64 partitions, multiply on the Vector engine.
  - output projection: lhsT = outT [128 (2 heads), 128 q] against the
    matching W_out rows, accumulated over head pairs -> y [q, DO].
  - emission is software-pipelined so the TensorE never starves (a stall
    >3.4us drops the HAM clock gate from 2.4GHz to 1.2GHz): attn@V matmuls
    of head h-1 and the previous tile's output projection are woven
    between score matmul groups as ACT-independent filler.
"""

import sys

for _p in ("/opt/trn_rl_repo", "/root/.axon_site/_ro/trn_rl_repo"):
    if _p not in sys.path:
        sys.path.append(_p)

import ml_dtypes
import numpy as np

import concourse.bass as bass
import concourse.tile as tile
from concourse import bacc, mybir

F32 = mybir.dt.float32
F32R = mybir.dt.float32r
BF16 = mybir.dt.bfloat16
AF = mybir.ActivationFunctionType

# Problem dims (hardcoded per the grading contract).
B, S, D = 4, 2048, 512
H, DH = 8, 64
INNER = H * DH
HL = 4                # heads per core
DO = D                # output dim
QT = 512              # query tile
SCALE = DH ** -0.5

N_CORES = 8
COMPUTE_DT = "bf16"   # "bf16" | "f32r"


def build_nc(S=S, D=D, HL=HL, DH=DH, DO=DO, QT=QT, n_cores=N_CORES,
             compute_dt=COMPUTE_DT):
    KB = S // 128         # k-token blocks
    DC = D // 128         # contraction chunks for the projections
    QKF = 2 * HL * DH     # q+k feature count per core
    MQK = QKF // 128      # qk feature blocks (2 heads each)
    VF = HL * DH          # v feature count per core
    NQT = S // QT         # query tiles
    SG = 2                # exp supergroup: PSUM banks per ACTIVATE
    NG = KB // SG         # score groups per head

    if compute_dt == "bf16":
        CDT = BF16
        in_dt = BF16

        def incast(ap):
            return ap
    else:
        CDT = F32R
        in_dt = F32

        def incast(ap):
            return ap.bitcast(F32R)

    nc = bacc.Bacc(
        "TRN2", target_bir_lowering=False, debug=False, num_devices=n_cores
    )
    xT = nc.dram_tensor("xT", [D, S], in_dt, kind="ExternalInput").ap()
    wqk = nc.dram_tensor("wqk", [D, QKF], in_dt, kind="ExternalInput").ap()
    wv = nc.dram_tensor("wv", [D, VF], in_dt, kind="ExternalInput").ap()
    wo = nc.dram_tensor("wo", [VF, DO], in_dt, kind="ExternalInput").ap()
    y = nc.dram_tensor("y", [S, DO], F32, kind="ExternalOutput").ap()

    with tile.TileContext(nc) as tc:
        with (
            tc.tile_pool(name="weights", bufs=1) as wpool,
            tc.tile_pool(name="big", bufs=1) as big,
        ):
            # ---- loads ----
            xT_sb = big.tile([128, DC, S], CDT)
            x_view = xT.rearrange("(c p) s -> c p s", p=128)
            for c in range(DC):
                for th in range(2):
                    sl = slice(th * (S // 2), (th + 1) * (S // 2))
                    nc.sync.dma_start(
                        out=xT_sb[:, c, sl], in_=incast(x_view[c][:, sl])
                    )
            wqk_sb = wpool.tile([128, DC, QKF], CDT)
            nc.sync.dma_start(
                out=wqk_sb, in_=incast(wqk.rearrange("(c p) f -> p c f", p=128))
            )
            wv_sb = wpool.tile([128, DC, VF], CDT)
            nc.sync.dma_start(
                out=wv_sb, in_=incast(wv.rearrange("(c p) f -> p c f", p=128))
            )
            wo_sb = wpool.tile([128, HL // 2, DO], CDT)
            nc.sync.dma_start(
                out=wo_sb, in_=incast(wo.rearrange("(c p) d -> p c d", p=128))
            )

            # ---- phase A: projections ----
            # qkT chunk h    = qT of head h  (real rows (h%2)*64..+64, rest 0)
            # qkT chunk HL+h = kT of head h  (same padding)
            qkT = big.tile([128, 2 * HL, S], CDT)
            nc.vector.memset(qkT, 0.0)
            vaug = big.tile([128, KB, HL, DH + 1], CDT)
            with tc.tile_pool(name="psA", bufs=3, space="PSUM") as psA:
                for m in range(MQK):
                    base = HL if m >= MQK // 2 else 0
                    hp = 2 * (m % (MQK // 2))
                    for t in range(S // 512):
                        sl = slice(t * 512, (t + 1) * 512)
                        ps = psA.tile([128, 512], F32, tag="qk")
                        for c in range(DC):
                            nc.tensor.matmul(
                                ps,
                                lhsT=wqk_sb[:, c, m * 128:(m + 1) * 128],
                                rhs=xT_sb[:, c, sl],
                                start=(c == 0),
                                stop=(c == DC - 1),
                            )
                        nc.scalar.copy(
                            out=qkT[0:64, base + hp, sl], in_=ps[0:64, :]
                        )
                        nc.vector.tensor_copy(
                            out=qkT[64:128, base + hp + 1, sl],
                            in_=ps[64:128, :],
                        )

                # V in natural [token, dh] layout, +1 ones column per head.
                ones_col = vaug[:, :, :, DH:DH + 1]
                nc.vector.memset(
                    ones_col.bitcast(F32) if CDT is F32R else ones_col, 1.0
                )
                for t in range(KB):
                    ps = psA.tile([128, VF], F32, tag="v")
                    for c in range(DC):
                        nc.tensor.matmul(
                            ps,
                            lhsT=xT_sb[:, c, t * 128:(t + 1) * 128],
                            rhs=wv_sb[:, c, :],
                            start=(c == 0),
                            stop=(c == DC - 1),
                        )
                    nc.scalar.copy(
                        out=vaug[:, t, :, 0:DH],
                        in_=ps.rearrange("p (h e) -> p h e", h=HL),
                    )

            # ---- phase B: attention + output projection (woven) ----
            with (
                tc.tile_pool(name="psS", bufs=2, space="PSUM") as psS,
                tc.tile_pool(name="psB2", bufs=4, space="PSUM") as psB2,
                tc.tile_pool(name="attnp", bufs=4) as attnp,
                tc.tile_pool(name="outp", bufs=2) as outp,
                tc.tile_pool(name="smalls", bufs=2) as smalls,
                tc.tile_pool(name="ysbp", bufs=2) as ysbp,
            ):
                # proj work left over from the previous q-tile: closures,
                # each emitting one PSUM accumulation + store.
                pending_proj = []

                def make_proj_units(outT, n):
                    units = []
                    for qb in range(QT // 128):
                        def unit(qb=qb, outT=outT, n=n):
                            yps = psB2.tile([128, DO], F32, tag="bank")
                            for c in range(HL // 2):
                                nc.tensor.matmul(
                                    yps,
                                    lhsT=outT[:, c, qb * 128:(qb + 1) * 128],
                                    rhs=wo_sb[:, c, :],
                                    start=(c == 0),
                                    stop=(c == HL // 2 - 1),
                                    skip_group_check=True,
                                )
                            ysb = ysbp.tile([128, DO], F32, tag="ysb")
                            nc.vector.tensor_copy(out=ysb, in_=yps)
                            nc.sync.dma_start(
                                out=y[n * QT + qb * 128:
                                      n * QT + (qb + 1) * 128, :],
                                in_=ysb,
                            )
                        units.append(unit)
                    return units

                for n in range(NQT):
                    outT = outp.tile([128, HL // 2, QT], CDT, tag="outT")
                    at = {}
                    avps = {}
                    avk = {h: 0 for h in range(HL)}

                    def score_unit(h, g, n=n, at=at):
                        if g == 0:
                            at[h] = attnp.tile([128, KB, QT], CDT, tag="attnT", name="at")
                        qs = qkT[:, h, n * QT:(n + 1) * QT]
                        ps = psS.tile([128, SG, 512], F32, tag="sc")
                        for i in range(SG):
                            kb = g * SG + i
                            nc.tensor.matmul(
                                ps[:, i, :],
                                lhsT=qkT[:, HL + h, kb * 128:(kb + 1) * 128],
                                rhs=qs,
                                skip_group_check=True,
                            )
                        nc.scalar.activation(
                            out=at[h][:, g * SG:(g + 1) * SG, :], in_=ps,
                            func=AF.Exp, scale=SCALE,
                        )

                    def normalize(h, outT=outT, avps=avps):
                        ps = avps[h]
                        rd = smalls.tile([DH + 1, QT], F32, tag="rd")
                        nc.vector.reciprocal(
                            rd[DH:DH + 1, :], ps[DH:DH + 1, :]
                        )
                        rb = smalls.tile([64, QT], F32, tag="rb")
                        nc.sync.dma_start(
                            out=rb.unsqueeze(1),
                            in_=rd[DH:DH + 1, None, :].broadcast_to(
                                [1, 64, QT]),
                        )
                        if h % 2 == 0:
                            nc.vector.tensor_mul(
                                outT[0:64, h // 2, :], ps[0:DH, :], rb
                            )
                        else:
                            ot = smalls.tile([64, QT], CDT, tag="ot")
                            nc.vector.tensor_mul(ot, ps[0:DH, :], rb)
                            nc.sync.dma_start(
                                out=outT[64:128, h // 2, :], in_=ot
                            )

                    def av_mms(h, cnt, at=at, avps=avps, avk=avk):
                        for _ in range(cnt):
                            kb = avk[h]
                            avk[h] = kb + 1
                            if kb == 0:
                                avps[h] = psB2.tile(
                                    [DH + 1, QT], F32, tag="bank", name="avp"
                                )
                            nc.tensor.matmul(
                                avps[h],
                                lhsT=vaug[:, kb, h, :],
                                rhs=at[h][:, kb, :],
                                start=(kb == 0),
                                stop=(kb == KB - 1),
                                skip_group_check=True,
                            )
                        if avk[h] == KB:
                            normalize(h)

                    for h in range(HL):
                        for g in range(NG):
                            score_unit(h, g)
                            if h == 0:
                                # weave the previous tile's projection into
                                # the first head's score stream (late, when
                                # its outT is surely normalized).
                                if pending_proj and g >= 5:
                                    pending_proj.pop(0)()
                            else:
                                if h == 1 and g == 0:
                                    while pending_proj:
                                        pending_proj.pop(0)()
                                av_mms(h - 1, 2)
                                if h == HL - 1 and g >= 1:
                                    av_mms(HL - 1, 2)
                    av_mms(HL - 1, 2)

                    pending_proj = make_proj_units(outT, n)

                for u in pending_proj:
                    u()

    nc.compile()
    return nc


def shard_inputs(x, W_qkv, W_out, compute_dt=COMPUTE_DT):
    """Full inputs -> list of 8 per-core input maps."""
    dt = ml_dtypes.bfloat16 if compute_dt == "bf16" else np.float32
    in_maps = []
    for c in range(N_CORES):
        b, g = divmod(c, 2)
        qcols = W_qkv[:, g * 256:(g + 1) * 256]
        kcols = W_qkv[:, INNER + g * 256:INNER + (g + 1) * 256]
        vcols = W_qkv[:, 2 * INNER + g * 256:2 * INNER + (g + 1) * 256]
        in_maps.append({
            "xT": np.ascontiguousarray(x[b].T).astype(dt),
            "wqk": np.ascontiguousarray(
                np.concatenate([qcols, kcols], axis=1)).astype(dt),
            "wv": np.ascontiguousarray(vcols).astype(dt),
            "wo": np.ascontiguousarray(
                W_out[g * 256:(g + 1) * 256, :]).astype(dt),
        })
    return in_maps


def gather_output(ys, b_out):
    out = np.empty((B, S, DO), np.float32)
    for b in range(B):
        out[b] = ys[2 * b] + ys[2 * b + 1]
        out[b] += b_out
    return out


_NC_CACHE = {}


def _get_nc():
    if "nc" not in _NC_CACHE:
        _NC_CACHE["nc"] = build_nc()
    return _NC_CACHE["nc"]


def kernel(**inputs):
    x = np.asarray(inputs["x"], np.float32)
    W_qkv = np.asarray(inputs["W_qkv"], np.float32)
    W_out = np.asarray(inputs["W_out"], np.float32)
    b_out = np.asarray(inputs["b_out"], np.float32)

    from concourse.bass_utils import run_bass_kernel_spmd

    nc = _get_nc()
    in_maps = shard_inputs(x, W_qkv, W_out)
    res = run_bass_kernel_spmd(nc, in_maps, core_ids=list(range(N_CORES)))
    ys = [r["y"] for r in res.results]
    return gather_output(ys, b_out)


# revision 16
# speedup vs baseline: 1.1241x; 1.1241x over previous
"""Multi-head attention (B=4, S=2048, D=512, H=8, DH=64) on 8 TRN2 NeuronCores.

Sharding: core c handles batch b = c//2 and head-group g = c%2 (4 of the 8
heads).  Each core computes its QKV projection (columns of W_qkv for its
heads), attention for its 4 heads, and a partial output projection
(rows of W_out for its heads).  The host sums the two partials per batch
and adds the bias.

Per-core device layout (compute in bf16, fp32 PSUM accumulation):
  - host passes x[b] pre-transposed (xT [D, S]); the QKV projection then
    needs no on-device transpose: qkT[f, s] = sum_d wqk[d, f] * xT[d, s].
  - qT/kT are stored one head per 128-partition chunk with the unused 64
    partitions zeroed, so every matmul in the kernel runs in the same
    128x128 array mode (no TensorE mode-switch drains) and score matmuls
    contract over K=128 (the zero rows contribute nothing).
  - scores are computed transposed (scoresT [k, q]); exp(scale*s) is fused
    into the PSUM->SBUF copy on the Scalar engine, batched 2 PSUM banks at
    a time to amortize the ACTIVATE fixed overhead.
  - attn @ V uses V in natural [token, dh] layout augmented with a ones
    column: one PSUM accumulation produces outT_unnorm [dh, q] AND the
    softmax denominator row.
  - normalization: reciprocal of the denominator row, DMA-broadcast across
    64 partitions, multiply on the Vector engine.
  - output projection: lhsT = outT [128 (2 heads), 128 q] against the
    matching W_out rows, accumulated over head pairs -> y [q, DO].
  - emission is software-pipelined so the TensorE never starves (a stall
    >3.4us drops the HAM clock gate from 2.4GHz to 1.2GHz): attn@V matmuls
    of head h-1 and the previous tile's output projection are woven
    between score matmul groups as ACT-independent filler.
"""

import sys

for _p in ("/opt/trn_rl_repo", "/root/.axon_site/_ro/trn_rl_repo"):
    if _p not in sys.path:
        sys.path.append(_p)

import ml_dtypes
import numpy as np

import concourse.bass as bass
import concourse.tile as tile
from concourse import bacc, mybir

F32 = mybir.dt.float32
F32R = mybir.dt.float32r
BF16 = mybir.dt.bfloat16
AF = mybir.ActivationFunctionType

# Problem dims (hardcoded per the grading contract).
B, S, D = 4, 2048, 512
H, DH = 8, 64
INNER = H * DH
HL = 4                # heads per core
DO = D                # output dim
QT = 512              # query tile
SCALE = DH ** -0.5

N_CORES = 8
COMPUTE_DT = "bf16"   # "bf16" | "f32r"


def build_nc(S=S, D=D, HL=HL, DH=DH, DO=DO, QT=QT, n_cores=N_CORES,
             compute_dt=COMPUTE_DT):
    KB = S // 128         # k-token blocks
    DC = D // 128         # contraction chunks for the projections
    QKF = 2 * HL * DH     # q+k feature count per core
    MQK = QKF // 128      # qk feature blocks (2 heads each)
    VF = HL * DH          # v feature count per core
    NQT = S // QT         # query tiles
    SG = 2                # exp supergroup: PSUM banks per ACTIVATE
    NG = KB // SG         # score groups per head

    if compute_dt == "bf16":
        CDT = BF16
        in_dt = BF16

        def incast(ap):
            return ap
    else:
        CDT = F32R
        in_dt = F32

        def incast(ap):
            return ap.bitcast(F32R)

    nc = bacc.Bacc(
        "TRN2", target_bir_lowering=False, debug=False, num_devices=n_cores
    )
    xT = nc.dram_tensor("xT", [D, S], in_dt, kind="ExternalInput").ap()
    wqk = nc.dram_tensor("wqk", [D, QKF], in_dt, kind="ExternalInput").ap()
    wv = nc.dram_tensor("wv", [D, VF], in_dt, kind="ExternalInput").ap()
    wo = nc.dram_tensor("wo", [VF, DO], in_dt, kind="ExternalInput").ap()
    y = nc.dram_tensor("y", [S, DO], F32, kind="ExternalOutput").ap()

    with tile.TileContext(nc) as tc:
        with (
            tc.tile_pool(name="weights", bufs=1) as wpool,
            tc.tile_pool(name="big", bufs=1) as big,
        ):
            # ---- loads ----
            xT_sb = big.tile([128, DC, S], CDT)
            x_view = xT.rearrange("(c p) s -> c p s", p=128)
            for c in range(DC):
                for th in range(2):
                    sl = slice(th * (S // 2), (th + 1) * (S // 2))
                    nc.sync.dma_start(
                        out=xT_sb[:, c, sl], in_=incast(x_view[c][:, sl])
                    )
            wqk_sb = wpool.tile([128, DC, QKF], CDT)
            nc.sync.dma_start(
                out=wqk_sb, in_=incast(wqk.rearrange("(c p) f -> p c f", p=128))
            )
            wv_sb = wpool.tile([128, DC, VF], CDT)
            nc.sync.dma_start(
                out=wv_sb, in_=incast(wv.rearrange("(c p) f -> p c f", p=128))
            )
            wo_sb = wpool.tile([128, HL // 2, DO], CDT)
            nc.sync.dma_start(
                out=wo_sb, in_=incast(wo.rearrange("(c p) d -> p c d", p=128))
            )

            # ---- phase A: projections ----
            # qkT chunk h    = qT of head h  (real rows (h%2)*64..+64, rest 0)
            # qkT chunk HL+h = kT of head h  (same padding)
            qkT = big.tile([128, 2 * HL, S], CDT)
            nc.vector.memset(qkT, 0.0)
            vaug = big.tile([128, KB, HL, DH + 1], CDT)
            with tc.tile_pool(name="psA", bufs=3, space="PSUM") as psA:
                for m in range(MQK):
                    base = HL if m >= MQK // 2 else 0
                    hp = 2 * (m % (MQK // 2))
                    for t in range(S // 512):
                        sl = slice(t * 512, (t + 1) * 512)
                        ps = psA.tile([128, 512], F32, tag="qk")
                        for c in range(DC):
                            nc.tensor.matmul(
                                ps,
                                lhsT=wqk_sb[:, c, m * 128:(m + 1) * 128],
                                rhs=xT_sb[:, c, sl],
                                start=(c == 0),
                                stop=(c == DC - 1),
                            )
                        nc.scalar.copy(
                            out=qkT[0:64, base + hp, sl], in_=ps[0:64, :]
                        )
                        nc.vector.tensor_copy(
                            out=qkT[64:128, base + hp + 1, sl],
                            in_=ps[64:128, :],
                        )

                # V in natural [token, dh] layout, +1 ones column per head.
                ones_col = vaug[:, :, :, DH:DH + 1]
                nc.vector.memset(
                    ones_col.bitcast(F32) if CDT is F32R else ones_col, 1.0
                )
                for t in range(KB):
                    ps = psA.tile([128, VF], F32, tag="v")
                    for c in range(DC):
                        nc.tensor.matmul(
                            ps,
                            lhsT=xT_sb[:, c, t * 128:(t + 1) * 128],
                            rhs=wv_sb[:, c, :],
                            start=(c == 0),
                            stop=(c == DC - 1),
                        )
                    nc.scalar.copy(
                        out=vaug[:, t, :, 0:DH],
                        in_=ps.rearrange("p (h e) -> p h e", h=HL),
                    )

            # ---- phase B: attention + output projection (woven) ----
            with (
                tc.tile_pool(name="psS", bufs=2, space="PSUM") as psS,
                tc.tile_pool(name="psB2", bufs=4, space="PSUM") as psB2,
                tc.tile_pool(name="attnp", bufs=4) as attnp,
                tc.tile_pool(name="outp", bufs=2) as outp,
                tc.tile_pool(name="smalls", bufs=2) as smalls,
                tc.tile_pool(name="ysbp", bufs=2) as ysbp,
            ):
                # proj work left over from the previous q-tile: closures,
                # each emitting one PSUM accumulation + store.
                pending_proj = []

                def make_proj_units(outT, n):
                    units = []
                    for qb in range(QT // 128):
                        def unit(qb=qb, outT=outT, n=n):
                            yps = psB2.tile([128, DO], F32, tag="bank")
                            for c in range(HL // 2):
                                nc.tensor.matmul(
                                    yps,
                                    lhsT=outT[:, c, qb * 128:(qb + 1) * 128],
                                    rhs=wo_sb[:, c, :],
                                    start=(c == 0),
                                    stop=(c == HL // 2 - 1),
                                    skip_group_check=True,
                                )
                            ysb = ysbp.tile([128, DO], F32, tag="ysb")
                            nc.vector.tensor_copy(out=ysb, in_=yps)
                            nc.sync.dma_start(
                                out=y[n * QT + qb * 128:
                                      n * QT + (qb + 1) * 128, :],
                                in_=ysb,
                            )
                        units.append(unit)
                    return units

                carry = None  # last av chunks + normalize of prev tile's h3

                for n in range(NQT):
                    outT = outp.tile([128, HL // 2, QT], CDT, tag="outT")
                    at = {}
                    avps = {}
                    avk = {h: 0 for h in range(HL)}

                    def score_unit(h, g, n=n, at=at):
                        if g == 0:
                            at[h] = attnp.tile([128, KB, QT], CDT, tag="attnT", name="at")
                        qs = qkT[:, h, n * QT:(n + 1) * QT]
                        ps = psS.tile([128, SG, 512], F32, tag="sc")
                        for i in range(SG):
                            kb = g * SG + i
                            nc.tensor.matmul(
                                ps[:, i, :],
                                lhsT=qkT[:, HL + h, kb * 128:(kb + 1) * 128],
                                rhs=qs,
                                skip_group_check=True,
                            )
                        nc.scalar.activation(
                            out=at[h][:, g * SG:(g + 1) * SG, :], in_=ps,
                            func=AF.Exp, scale=SCALE,
                        )

                    def normalize(h, outT=outT, avps=avps):
                        ps = avps[h]
                        rd = smalls.tile([DH + 1, QT], F32, tag="rd")
                        nc.vector.reciprocal(
                            rd[DH:DH + 1, :], ps[DH:DH + 1, :]
                        )
                        rb = smalls.tile([64, QT], F32, tag="rb")
                        nc.sync.dma_start(
                            out=rb.unsqueeze(1),
                            in_=rd[DH:DH + 1, None, :].broadcast_to(
                                [1, 64, QT]),
                        )
                        if h % 2 == 0:
                            nc.vector.tensor_mul(
                                outT[0:64, h // 2, :], ps[0:DH, :], rb
                            )
                        else:
                            ot = smalls.tile([64, QT], CDT, tag="ot")
                            nc.vector.tensor_mul(ot, ps[0:DH, :], rb)
                            nc.sync.dma_start(
                                out=outT[64:128, h // 2, :], in_=ot
                            )

                    def av_mms(h, cnt, at=at, avps=avps, avk=avk,
                               normalize=normalize):
                        for _ in range(cnt):
                            kb = avk[h]
                            avk[h] = kb + 1
                            if kb == 0:
                                avps[h] = psB2.tile(
                                    [DH + 1, QT], F32, tag="bank", name="avp"
                                )
                            nc.tensor.matmul(
                                avps[h],
                                lhsT=vaug[:, kb, h, :],
                                rhs=at[h][:, kb, :],
                                start=(kb == 0),
                                stop=(kb == KB - 1),
                                skip_group_check=True,
                            )
                        if avk[h] == KB:
                            normalize(h)

                    # Weave: head h's attn@V follows its own scores one
                    # group behind; the last two chunks + normalize land on
                    # the next head's (or next tile's) first slot, so the
                    # four normalize chains spread evenly instead of
                    # bunching on the Vector engine.
                    for h in range(HL):
                        for g in range(NG):
                            score_unit(h, g)
                            if g == 0:
                                if h == 0:
                                    if carry is not None:
                                        carry()
                                        carry = None
                                else:
                                    av_mms(h - 1, 2)
                                if h == 1:
                                    while pending_proj:
                                        pending_proj.pop(0)()
                            else:
                                if h == 0 and pending_proj and g >= NG - 2:
                                    pending_proj.pop(0)()
                                av_mms(h, 2)

                    def make_carry(av_mms=av_mms):
                        return lambda: av_mms(HL - 1, 2)

                    carry = make_carry()
                    pending_proj = make_proj_units(outT, n)

                if carry is not None:
                    carry()

                for u in pending_proj:
                    u()

    nc.compile()
    return nc


def shard_inputs(x, W_qkv, W_out, compute_dt=COMPUTE_DT):
    """Full inputs -> list of 8 per-core input maps."""
    dt = ml_dtypes.bfloat16 if compute_dt == "bf16" else np.float32
    in_maps = []
    for c in range(N_CORES):
        b, g = divmod(c, 2)
        qcols = W_qkv[:, g * 256:(g + 1) * 256]
        kcols = W_qkv[:, INNER + g * 256:INNER + (g + 1) * 256]
        vcols = W_qkv[:, 2 * INNER + g * 256:2 * INNER + (g + 1) * 256]
        in_maps.append({
            "xT": np.ascontiguousarray(x[b].T).astype(dt),
            "wqk": np.ascontiguousarray(
                np.concatenate([qcols, kcols], axis=1)).astype(dt),
            "wv": np.ascontiguousarray(vcols).astype(dt),
            "wo": np.ascontiguousarray(
                W_out[g * 256:(g + 1) * 256, :]).astype(dt),
        })
    return in_maps


def gather_output(ys, b_out):
    out = np.empty((B, S, DO), np.float32)
    for b in range(B):
        out[b] = ys[2 * b] + ys[2 * b + 1]
        out[b] += b_out
    return out


_NC_CACHE = {}


def _get_nc():
    if "nc" not in _NC_CACHE:
        _NC_CACHE["nc"] = build_nc()
    return _NC_CACHE["nc"]


def kernel(**inputs):
    x = np.asarray(inputs["x"], np.float32)
    W_qkv = np.asarray(inputs["W_qkv"], np.float32)
    W_out = np.asarray(inputs["W_out"], np.float32)
    b_out = np.asarray(inputs["b_out"], np.float32)

    from concourse.bass_utils import run_bass_kernel_spmd

    nc = _get_nc()
    in_maps = shard_inputs(x, W_qkv, W_out)
    res = run_bass_kernel_spmd(nc, in_maps, core_ids=list(range(N_CORES)))
    ys = [r["y"] for r in res.results]
    return gather_output(ys, b_out)


# revision 18
# speedup vs baseline: 1.2951x; 1.1521x over previous
"""Multi-head attention (B=4, S=2048, D=512, H=8, DH=64) on 8 TRN2 NeuronCores.

Sharding: core c handles batch b = c//2 and head-group g = c%2 (4 of the 8
heads).  Each core computes its QKV projection (columns of W_qkv for its
heads), attention for its 4 heads, and a partial output projection
(rows of W_out for its heads).  The host sums the two partials per batch
and adds the bias.

Per-core device layout (compute in bf16, fp32 PSUM accumulation):
  - host passes x[b] pre-transposed (xT [D, S]); the QKV projection then
    needs no on-device transpose: qkT[f, s] = sum_d wqk[d, f] * xT[d, s].
  - qT/kT are stored one head per 128-partition chunk with the unused 64
    partitions zeroed, so every matmul in the kernel runs in the same
    128x128 array mode (no TensorE mode-switch drains) and score matmuls
    contract over K=128 (the zero rows contribute nothing).
  - scores are computed transposed (scoresT [k, q]); exp(scale*s) is fused
    into the PSUM->SBUF copy on the Scalar engine, batched 2 PSUM banks at
    a time to amortize the ACTIVATE fixed overhead.
  - attn @ V uses V in natural [token, dh] layout augmented with a ones
    column: one PSUM accumulation produces outT_unnorm [dh, q] AND the
    softmax denominator row.
  - normalization: reciprocal of the denominator row, DMA-broadcast across
    64 partitions, multiply on the Vector engine.
  - output projection: lhsT = outT [128 (2 heads), 128 q] against the
    matching W_out rows, accumulated over head pairs -> y [q, DO].
  - emission is software-pipelined so the TensorE never starves (a stall
    >3.4us drops the HAM clock gate from 2.4GHz to 1.2GHz): attn@V matmuls
    of head h-1 and the previous tile's output projection are woven
    between score matmul groups as ACT-independent filler.
"""

import sys

for _p in ("/opt/trn_rl_repo", "/root/.axon_site/_ro/trn_rl_repo"):
    if _p not in sys.path:
        sys.path.append(_p)

import ml_dtypes
import numpy as np

import concourse.bass as bass
import concourse.tile as tile
from concourse import bacc, mybir

F32 = mybir.dt.float32
F32R = mybir.dt.float32r
BF16 = mybir.dt.bfloat16
AF = mybir.ActivationFunctionType

# Problem dims (hardcoded per the grading contract).
B, S, D = 4, 2048, 512
H, DH = 8, 64
INNER = H * DH
HL = 4                # heads per core
DO = D                # output dim
QT = 512              # query tile
SCALE = DH ** -0.5

N_CORES = 8
COMPUTE_DT = "bf16"   # "bf16" | "f32r"


def build_nc(S=S, D=D, HL=HL, DH=DH, DO=DO, QT=QT, n_cores=N_CORES,
             compute_dt=COMPUTE_DT):
    KB = S // 128         # k-token blocks
    DC = D // 128         # contraction chunks for the projections
    QKF = 2 * HL * DH     # q+k feature count per core
    MQK = QKF // 128      # qk feature blocks (2 heads each)
    VF = HL * DH          # v feature count per core
    NQT = S // QT         # query tiles
    SG = 2                # exp supergroup: PSUM banks per ACTIVATE
    NG = KB // SG         # score groups per head

    if compute_dt == "bf16":
        CDT = BF16
        in_dt = BF16

        def incast(ap):
            return ap
    else:
        CDT = F32R
        in_dt = F32

        def incast(ap):
            return ap.bitcast(F32R)

    nc = bacc.Bacc(
        "TRN2", target_bir_lowering=False, debug=False, num_devices=n_cores
    )
    xT = nc.dram_tensor("xT", [D, S], in_dt, kind="ExternalInput").ap()
    wqk = nc.dram_tensor("wqk", [D, QKF], in_dt, kind="ExternalInput").ap()
    wv = nc.dram_tensor("wv", [D, VF], in_dt, kind="ExternalInput").ap()
    wo = nc.dram_tensor("wo", [VF, DO], in_dt, kind="ExternalInput").ap()
    y = nc.dram_tensor("y", [S, DO], F32, kind="ExternalOutput").ap()

    with tile.TileContext(nc) as tc:
        with (
            tc.tile_pool(name="weights", bufs=1) as wpool,
            tc.tile_pool(name="big", bufs=1) as big,
        ):
            # ---- loads ----
            xT_sb = big.tile([128, DC, S], CDT)
            x_view = xT.rearrange("(c p) s -> c p s", p=128)
            for c in range(DC):
                for th in range(2):
                    sl = slice(th * (S // 2), (th + 1) * (S // 2))
                    nc.sync.dma_start(
                        out=xT_sb[:, c, sl], in_=incast(x_view[c][:, sl])
                    )
            wqk_sb = wpool.tile([128, DC, QKF], CDT)
            nc.sync.dma_start(
                out=wqk_sb, in_=incast(wqk.rearrange("(c p) f -> p c f", p=128))
            )
            wv_sb = wpool.tile([128, DC, VF], CDT)
            nc.sync.dma_start(
                out=wv_sb, in_=incast(wv.rearrange("(c p) f -> p c f", p=128))
            )
            wo_sb = wpool.tile([128, HL // 2, DO], CDT)
            nc.sync.dma_start(
                out=wo_sb, in_=incast(wo.rearrange("(c p) d -> p c d", p=128))
            )

            # ---- phase A: projections ----
            # qkT chunk h    = qT of head h  (real rows (h%2)*64..+64, rest 0)
            # qkT chunk HL+h = kT of head h  (same padding)
            qkT = big.tile([128, 2 * HL, S], CDT)
            nc.vector.memset(qkT, 0.0)
            vaug = big.tile([128, KB, HL, DH + 1], CDT)
            with tc.tile_pool(name="psA", bufs=3, space="PSUM") as psA:
                for m in range(MQK):
                    base = HL if m >= MQK // 2 else 0
                    hp = 2 * (m % (MQK // 2))
                    for t in range(S // 512):
                        sl = slice(t * 512, (t + 1) * 512)
                        ps = psA.tile([128, 512], F32, tag="qk")
                        for c in range(DC):
                            nc.tensor.matmul(
                                ps,
                                lhsT=wqk_sb[:, c, m * 128:(m + 1) * 128],
                                rhs=xT_sb[:, c, sl],
                                start=(c == 0),
                                stop=(c == DC - 1),
                            )
                        nc.scalar.copy(
                            out=qkT[0:64, base + hp, sl], in_=ps[0:64, :]
                        )
                        nc.vector.tensor_copy(
                            out=qkT[64:128, base + hp + 1, sl],
                            in_=ps[64:128, :],
                        )

                # V in natural [token, dh] layout, +1 ones column per head.
                ones_col = vaug[:, :, :, DH:DH + 1]
                nc.vector.memset(
                    ones_col.bitcast(F32) if CDT is F32R else ones_col, 1.0
                )
                for t in range(KB):
                    ps = psA.tile([128, VF], F32, tag="v")
                    for c in range(DC):
                        nc.tensor.matmul(
                            ps,
                            lhsT=xT_sb[:, c, t * 128:(t + 1) * 128],
                            rhs=wv_sb[:, c, :],
                            start=(c == 0),
                            stop=(c == DC - 1),
                        )
                    nc.scalar.copy(
                        out=vaug[:, t, :, 0:DH],
                        in_=ps.rearrange("p (h e) -> p h e", h=HL),
                    )

            # ---- phase B: attention + output projection (woven) ----
            with (
                tc.tile_pool(name="psS", bufs=2, space="PSUM") as psS,
                tc.tile_pool(name="psB2", bufs=4, space="PSUM") as psB2,
                tc.tile_pool(name="attnp", bufs=4) as attnp,
                tc.tile_pool(name="outp", bufs=2) as outp,
                tc.tile_pool(name="smalls", bufs=2) as smalls,
                tc.tile_pool(name="ysbp", bufs=2) as ysbp,
            ):
                # proj work left over from the previous q-tile: closures,
                # each emitting one PSUM accumulation + store.
                pending_proj = []

                def make_proj_units(outT, n):
                    units = []
                    for qb in range(QT // 128):
                        def unit(qb=qb, outT=outT, n=n):
                            yps = psB2.tile([128, DO], F32, tag="bank")
                            for c in range(HL // 2):
                                nc.tensor.matmul(
                                    yps,
                                    lhsT=outT[:, c, qb * 128:(qb + 1) * 128],
                                    rhs=wo_sb[:, c, :],
                                    start=(c == 0),
                                    stop=(c == HL // 2 - 1),
                                    skip_group_check=True,
                                )
                            ysb = ysbp.tile([128, DO], F32, tag="ysb")
                            nc.vector.tensor_copy(out=ysb, in_=yps)
                            nc.sync.dma_start(
                                out=y[n * QT + qb * 128:
                                      n * QT + (qb + 1) * 128, :],
                                in_=ysb,
                            )
                        units.append(unit)
                    return units

                carry = None  # last av chunks + normalize of prev tile's h3

                for n in range(NQT):
                    outT = outp.tile([128, HL // 2, QT], CDT, tag="outT")
                    at = {}
                    avps = {}
                    avk = {h: 0 for h in range(HL)}

                    def score_unit(h, g, n=n, at=at):
                        if g == 0:
                            at[h] = attnp.tile([128, KB, QT], CDT, tag="attnT", name="at")
                        qs = qkT[:, h, n * QT:(n + 1) * QT]
                        ps = psS.tile([128, SG, 512], F32, tag="sc")
                        for i in range(SG):
                            kb = g * SG + i
                            nc.tensor.matmul(
                                ps[:, i, :],
                                lhsT=qkT[:, HL + h, kb * 128:(kb + 1) * 128],
                                rhs=qs,
                                skip_group_check=True,
                            )
                        nc.scalar.activation(
                            out=at[h][:, g * SG:(g + 1) * SG, :], in_=ps,
                            func=AF.Exp, scale=SCALE,
                        )

                    def normalize(h, outT=outT, avps=avps):
                        ps = avps[h]
                        rd = smalls.tile([DH + 1, QT], F32, tag="rd")
                        nc.vector.reciprocal(
                            rd[DH:DH + 1, :], ps[DH:DH + 1, :]
                        )
                        rd0 = smalls.tile([1, QT], F32, tag="rd0")
                        nc.sync.dma_start(out=rd0, in_=rd[DH:DH + 1, :])
                        rb = smalls.tile([64, QT], F32, tag="rb")
                        nc.gpsimd.partition_broadcast(rb, rd0, channels=64)
                        if h % 2 == 0:
                            nc.vector.tensor_mul(
                                outT[0:64, h // 2, :], ps[0:DH, :], rb
                            )
                        else:
                            ot = smalls.tile([64, QT], CDT, tag="ot")
                            nc.vector.tensor_mul(ot, ps[0:DH, :], rb)
                            nc.sync.dma_start(
                                out=outT[64:128, h // 2, :], in_=ot
                            )

                    def av_mms(h, cnt, at=at, avps=avps, avk=avk,
                               normalize=normalize):
                        for _ in range(cnt):
                            kb = avk[h]
                            avk[h] = kb + 1
                            if kb == 0:
                                avps[h] = psB2.tile(
                                    [DH + 1, QT], F32, tag="bank", name="avp"
                                )
                            nc.tensor.matmul(
                                avps[h],
                                lhsT=vaug[:, kb, h, :],
                                rhs=at[h][:, kb, :],
                                start=(kb == 0),
                                stop=(kb == KB - 1),
                                skip_group_check=True,
                            )
                        if avk[h] == KB:
                            normalize(h)

                    # Weave: head h's attn@V follows its own scores one
                    # group behind; the last two chunks + normalize land on
                    # the next head's (or next tile's) first slot, so the
                    # four normalize chains spread evenly instead of
                    # bunching on the Vector engine.
                    for h in range(HL):
                        for g in range(NG):
                            score_unit(h, g)
                            if g == 0:
                                if h == 0:
                                    if carry is not None:
                                        carry()
                                        carry = None
                                else:
                                    av_mms(h - 1, 2)
                                if h == 1:
                                    while pending_proj:
                                        pending_proj.pop(0)()
                            else:
                                if h == 0 and pending_proj and g >= NG - 2:
                                    pending_proj.pop(0)()
                                av_mms(h, 2)

                    def make_carry(av_mms=av_mms):
                        return lambda: av_mms(HL - 1, 2)

                    carry = make_carry()
                    pending_proj = make_proj_units(outT, n)

                if carry is not None:
                    carry()

                for u in pending_proj:
                    u()

    nc.compile()
    return nc


def shard_inputs(x, W_qkv, W_out, compute_dt=COMPUTE_DT):
    """Full inputs -> list of 8 per-core input maps."""
    dt = ml_dtypes.bfloat16 if compute_dt == "bf16" else np.float32
    in_maps = []
    for c in range(N_CORES):
        b, g = divmod(c, 2)
        qcols = W_qkv[:, g * 256:(g + 1) * 256]
        kcols = W_qkv[:, INNER + g * 256:INNER + (g + 1) * 256]
        vcols = W_qkv[:, 2 * INNER + g * 256:2 * INNER + (g + 1) * 256]
        in_maps.append({
            "xT": np.ascontiguousarray(x[b].T).astype(dt),
            "wqk": np.ascontiguousarray(
                np.concatenate([qcols, kcols], axis=1)).astype(dt),
            "wv": np.ascontiguousarray(vcols).astype(dt),
            "wo": np.ascontiguousarray(
                W_out[g * 256:(g + 1) * 256, :]).astype(dt),
        })
    return in_maps


def gather_output(ys, b_out):
    out = np.empty((B, S, DO), np.float32)
    for b in range(B):
        out[b] = ys[2 * b] + ys[2 * b + 1]
        out[b] += b_out
    return out


_NC_CACHE = {}


def _get_nc():
    if "nc" not in _NC_CACHE:
        _NC_CACHE["nc"] = build_nc()
    return _NC_CACHE["nc"]


def kernel(**inputs):
    x = np.asarray(inputs["x"], np.float32)
    W_qkv = np.asarray(inputs["W_qkv"], np.float32)
    W_out = np.asarray(inputs["W_out"], np.float32)
    b_out = np.asarray(inputs["b_out"], np.float32)

    from concourse.bass_utils import run_bass_kernel_spmd

    nc = _get_nc()
    in_maps = shard_inputs(x, W_qkv, W_out)
    res = run_bass_kernel_spmd(nc, in_maps, core_ids=list(range(N_CORES)))
    ys = [r["y"] for r in res.results]
    return gather_output(ys, b_out)


# revision 19
# speedup vs baseline: 1.3240x; 1.0223x over previous
"""Multi-head attention (B=4, S=2048, D=512, H=8, DH=64) on 8 TRN2 NeuronCores.

Sharding: core c handles batch b = c//2 and head-group g = c%2 (4 of the 8
heads).  Each core computes its QKV projection (columns of W_qkv for its
heads), attention for its 4 heads, and a partial output projection
(rows of W_out for its heads).  The host sums the two partials per batch
and adds the bias.

Per-core device layout (compute in bf16, fp32 PSUM accumulation):
  - host passes x[b] pre-transposed (xT [D, S]); the QKV projection then
    needs no on-device transpose: qkT[f, s] = sum_d wqk[d, f] * xT[d, s].
  - qT/kT are stored one head per 128-partition chunk with the unused 64
    partitions zeroed, so every matmul in the kernel runs in the same
    128x128 array mode (no TensorE mode-switch drains) and score matmuls
    contract over K=128 (the zero rows contribute nothing).
  - scores are computed transposed (scoresT [k, q]); exp(scale*s) is fused
    into the PSUM->SBUF copy on the Scalar engine, batched 2 PSUM banks at
    a time to amortize the ACTIVATE fixed overhead.
  - attn @ V uses V in natural [token, dh] layout augmented with a ones
    column: one PSUM accumulation produces outT_unnorm [dh, q] AND the
    softmax denominator row.
  - normalization: reciprocal of the denominator row, DMA-broadcast across
    64 partitions, multiply on the Vector engine.
  - output projection: lhsT = outT [128 (2 heads), 128 q] against the
    matching W_out rows, accumulated over head pairs -> y [q, DO].
  - emission is software-pipelined so the TensorE never starves (a stall
    >3.4us drops the HAM clock gate from 2.4GHz to 1.2GHz): attn@V matmuls
    of head h-1 and the previous tile's output projection are woven
    between score matmul groups as ACT-independent filler.
"""

import sys

for _p in ("/opt/trn_rl_repo", "/root/.axon_site/_ro/trn_rl_repo"):
    if _p not in sys.path:
        sys.path.append(_p)

import ml_dtypes
import numpy as np

import concourse.bass as bass
import concourse.tile as tile
from concourse import bacc, mybir

F32 = mybir.dt.float32
F32R = mybir.dt.float32r
BF16 = mybir.dt.bfloat16
AF = mybir.ActivationFunctionType

# Problem dims (hardcoded per the grading contract).
B, S, D = 4, 2048, 512
H, DH = 8, 64
INNER = H * DH
HL = 4                # heads per core
DO = D                # output dim
QT = 512              # query tile
SCALE = DH ** -0.5

N_CORES = 8
COMPUTE_DT = "bf16"   # "bf16" | "f32r"


def build_nc(S=S, D=D, HL=HL, DH=DH, DO=DO, QT=QT, n_cores=N_CORES,
             compute_dt=COMPUTE_DT):
    KB = S // 128         # k-token blocks
    DC = D // 128         # contraction chunks for the projections
    QKF = 2 * HL * DH     # q+k feature count per core
    MQK = QKF // 128      # qk feature blocks (2 heads each)
    VF = HL * DH          # v feature count per core
    NQT = S // QT         # query tiles
    SG = 2                # exp supergroup: PSUM banks per ACTIVATE
    NG = KB // SG         # score groups per head

    if compute_dt == "bf16":
        CDT = BF16
        in_dt = BF16

        def incast(ap):
            return ap
    else:
        CDT = F32R
        in_dt = F32

        def incast(ap):
            return ap.bitcast(F32R)

    nc = bacc.Bacc(
        "TRN2", target_bir_lowering=False, debug=False, num_devices=n_cores
    )
    xT = nc.dram_tensor("xT", [D, S], in_dt, kind="ExternalInput").ap()
    wqk = nc.dram_tensor("wqk", [D, QKF], in_dt, kind="ExternalInput").ap()
    wv = nc.dram_tensor("wv", [D, VF], in_dt, kind="ExternalInput").ap()
    wo = nc.dram_tensor("wo", [VF, DO], in_dt, kind="ExternalInput").ap()
    y = nc.dram_tensor("y", [S, DO], F32, kind="ExternalOutput").ap()

    with tile.TileContext(nc) as tc:
        with (
            tc.tile_pool(name="weights", bufs=1) as wpool,
            tc.tile_pool(name="big", bufs=1) as big,
        ):
            # ---- loads ----
            wqk_sb = wpool.tile([128, DC, QKF], CDT)
            nc.sync.dma_start(
                out=wqk_sb, in_=incast(wqk.rearrange("(c p) f -> p c f", p=128))
            )
            xT_sb = big.tile([128, DC, S], CDT)
            x_view = xT.rearrange("(c p) s -> c p s", p=128)
            for c in range(DC):
                sl = slice(0, S // 2)
                nc.sync.dma_start(
                    out=xT_sb[:, c, sl], in_=incast(x_view[c][:, sl])
                )
            wv_sb = wpool.tile([128, DC, VF], CDT)
            nc.sync.dma_start(
                out=wv_sb, in_=incast(wv.rearrange("(c p) f -> p c f", p=128))
            )
            wo_sb = wpool.tile([128, HL // 2, DO], CDT)
            nc.sync.dma_start(
                out=wo_sb, in_=incast(wo.rearrange("(c p) d -> p c d", p=128))
            )
            for c in range(DC):
                sl = slice(S // 2, S)
                nc.sync.dma_start(
                    out=xT_sb[:, c, sl], in_=incast(x_view[c][:, sl])
                )

            # ---- phase A: projections ----
            # qkT chunk h    = qT of head h  (real rows (h%2)*64..+64, rest 0)
            # qkT chunk HL+h = kT of head h  (same padding)
            qkT = big.tile([128, 2 * HL, S], CDT)
            nc.vector.memset(qkT, 0.0)
            vaug = big.tile([128, KB, HL, DH + 1], CDT)
            with tc.tile_pool(name="psA", bufs=3, space="PSUM") as psA:
                for m in range(MQK):
                    base = HL if m >= MQK // 2 else 0
                    hp = 2 * (m % (MQK // 2))
                    for t in range(S // 512):
                        sl = slice(t * 512, (t + 1) * 512)
                        ps = psA.tile([128, 512], F32, tag="qk")
                        for c in range(DC):
                            nc.tensor.matmul(
                                ps,
                                lhsT=wqk_sb[:, c, m * 128:(m + 1) * 128],
                                rhs=xT_sb[:, c, sl],
                                start=(c == 0),
                                stop=(c == DC - 1),
                            )
                        nc.scalar.copy(
                            out=qkT[0:64, base + hp, sl], in_=ps[0:64, :]
                        )
                        nc.vector.tensor_copy(
                            out=qkT[64:128, base + hp + 1, sl],
                            in_=ps[64:128, :],
                        )

                # V in natural [token, dh] layout, +1 ones column per head.
                ones_col = vaug[:, :, :, DH:DH + 1]
                nc.vector.memset(
                    ones_col.bitcast(F32) if CDT is F32R else ones_col, 1.0
                )
                for t in range(KB):
                    ps = psA.tile([128, VF], F32, tag="v")
                    for c in range(DC):
                        nc.tensor.matmul(
                            ps,
                            lhsT=xT_sb[:, c, t * 128:(t + 1) * 128],
                            rhs=wv_sb[:, c, :],
                            start=(c == 0),
                            stop=(c == DC - 1),
                        )
                    nc.scalar.copy(
                        out=vaug[:, t, :, 0:DH],
                        in_=ps.rearrange("p (h e) -> p h e", h=HL),
                    )

            # ---- phase B: attention + output projection (woven) ----
            with (
                tc.tile_pool(name="psS", bufs=2, space="PSUM") as psS,
                tc.tile_pool(name="psB2", bufs=4, space="PSUM") as psB2,
                tc.tile_pool(name="attnp", bufs=4) as attnp,
                tc.tile_pool(name="outp", bufs=2) as outp,
                tc.tile_pool(name="smalls", bufs=2) as smalls,
                tc.tile_pool(name="ysbp", bufs=2) as ysbp,
            ):
                # proj work left over from the previous q-tile: closures,
                # each emitting one PSUM accumulation + store.
                pending_proj = []

                def make_proj_units(outT, n):
                    units = []
                    for qb in range(QT // 128):
                        def unit(qb=qb, outT=outT, n=n):
                            yps = psB2.tile([128, DO], F32, tag="bank")
                            for c in range(HL // 2):
                                nc.tensor.matmul(
                                    yps,
                                    lhsT=outT[:, c, qb * 128:(qb + 1) * 128],
                                    rhs=wo_sb[:, c, :],
                                    start=(c == 0),
                                    stop=(c == HL // 2 - 1),
                                    skip_group_check=True,
                                )
                            ysb = ysbp.tile([128, DO], F32, tag="ysb")
                            nc.vector.tensor_copy(out=ysb, in_=yps)
                            nc.sync.dma_start(
                                out=y[n * QT + qb * 128:
                                      n * QT + (qb + 1) * 128, :],
                                in_=ysb,
                            )
                        units.append(unit)
                    return units

                carry = None  # last av chunks + normalize of prev tile's h3

                for n in range(NQT):
                    outT = outp.tile([128, HL // 2, QT], CDT, tag="outT")
                    at = {}
                    avps = {}
                    avk = {h: 0 for h in range(HL)}

                    def score_unit(h, g, n=n, at=at):
                        if g == 0:
                            at[h] = attnp.tile([128, KB, QT], CDT, tag="attnT", name="at")
                        qs = qkT[:, h, n * QT:(n + 1) * QT]
                        ps = psS.tile([128, SG, 512], F32, tag="sc")
                        for i in range(SG):
                            kb = g * SG + i
                            nc.tensor.matmul(
                                ps[:, i, :],
                                lhsT=qkT[:, HL + h, kb * 128:(kb + 1) * 128],
                                rhs=qs,
                                skip_group_check=True,
                            )
                        nc.scalar.activation(
                            out=at[h][:, g * SG:(g + 1) * SG, :], in_=ps,
                            func=AF.Exp, scale=SCALE,
                        )

                    def normalize(h, outT=outT, avps=avps):
                        ps = avps[h]
                        rd = smalls.tile([DH + 1, QT], F32, tag="rd")
                        nc.vector.reciprocal(
                            rd[DH:DH + 1, :], ps[DH:DH + 1, :]
                        )
                        rd0 = smalls.tile([1, QT], F32, tag="rd0")
                        nc.sync.dma_start(out=rd0, in_=rd[DH:DH + 1, :])
                        rb = smalls.tile([64, QT], F32, tag="rb")
                        nc.gpsimd.partition_broadcast(rb, rd0, channels=64)
                        if h % 2 == 0:
                            nc.vector.tensor_mul(
                                outT[0:64, h // 2, :], ps[0:DH, :], rb
                            )
                        else:
                            ot = smalls.tile([64, QT], CDT, tag="ot")
                            nc.vector.tensor_mul(ot, ps[0:DH, :], rb)
                            nc.sync.dma_start(
                                out=outT[64:128, h // 2, :], in_=ot
                            )

                    def av_mms(h, cnt, at=at, avps=avps, avk=avk,
                               normalize=normalize):
                        for _ in range(cnt):
                            kb = avk[h]
                            avk[h] = kb + 1
                            if kb == 0:
                                avps[h] = psB2.tile(
                                    [DH + 1, QT], F32, tag="bank", name="avp"
                                )
                            nc.tensor.matmul(
                                avps[h],
                                lhsT=vaug[:, kb, h, :],
                                rhs=at[h][:, kb, :],
                                start=(kb == 0),
                                stop=(kb == KB - 1),
                                skip_group_check=True,
                            )
                        if avk[h] == KB:
                            normalize(h)

                    # Weave: head h's attn@V follows its own scores one
                    # group behind; the last two chunks + normalize land on
                    # the next head's (or next tile's) first slot, so the
                    # four normalize chains spread evenly instead of
                    # bunching on the Vector engine.
                    for h in range(HL):
                        for g in range(NG):
                            score_unit(h, g)
                            if g == 0:
                                if h == 0:
                                    if carry is not None:
                                        carry()
                                        carry = None
                                else:
                                    av_mms(h - 1, 2)
                            else:
                                av_mms(h, 2)
                            if h == 1 and pending_proj:
                                pending_proj.pop(0)()

                    def make_carry(av_mms=av_mms):
                        return lambda: av_mms(HL - 1, 2)

                    carry = make_carry()
                    pending_proj = make_proj_units(outT, n)

                if carry is not None:
                    carry()

                for u in pending_proj:
                    u()

    nc.compile()
    return nc


def shard_inputs(x, W_qkv, W_out, compute_dt=COMPUTE_DT):
    """Full inputs -> list of 8 per-core input maps."""
    dt = ml_dtypes.bfloat16 if compute_dt == "bf16" else np.float32
    in_maps = []
    for c in range(N_CORES):
        b, g = divmod(c, 2)
        qcols = W_qkv[:, g * 256:(g + 1) * 256]
        kcols = W_qkv[:, INNER + g * 256:INNER + (g + 1) * 256]
        vcols = W_qkv[:, 2 * INNER + g * 256:2 * INNER + (g + 1) * 256]
        in_maps.append({
            "xT": np.ascontiguousarray(x[b].T).astype(dt),
            "wqk": np.ascontiguousarray(
                np.concatenate([qcols, kcols], axis=1)).astype(dt),
            "wv": np.ascontiguousarray(vcols).astype(dt),
            "wo": np.ascontiguousarray(
                W_out[g * 256:(g + 1) * 256, :]).astype(dt),
        })
    return in_maps


def gather_output(ys, b_out):
    out = np.empty((B, S, DO), np.float32)
    for b in range(B):
        out[b] = ys[2 * b] + ys[2 * b + 1]
        out[b] += b_out
    return out


_NC_CACHE = {}


def _get_nc():
    if "nc" not in _NC_CACHE:
        _NC_CACHE["nc"] = build_nc()
    return _NC_CACHE["nc"]


def kernel(**inputs):
    x = np.asarray(inputs["x"], np.float32)
    W_qkv = np.asarray(inputs["W_qkv"], np.float32)
    W_out = np.asarray(inputs["W_out"], np.float32)
    b_out = np.asarray(inputs["b_out"], np.float32)

    from concourse.bass_utils import run_bass_kernel_spmd

    nc = _get_nc()
    in_maps = shard_inputs(x, W_qkv, W_out)
    res = run_bass_kernel_spmd(nc, in_maps, core_ids=list(range(N_CORES)))
    ys = [r["y"] for r in res.results]
    return gather_output(ys, b_out)
